# revision 12
# baseline (speedup 1.0000x reference)
"""Expert-parallel MoE FFN kernel for Trainium2 (8 NeuronCores).

Strategy (sharding_hint: expert-parallel):
  - Host computes the gate in fp32 (softmax -> top-2 -> renormalize) and
    dispatches tokens to experts (the "all-to-all" happens in host staging,
    which is legal because kernel() receives FULL inputs).
  - Core e holds expert e's weights (bf16) and processes its gathered tokens
    (padded to a static capacity C) through the FFN:
        Y = (gelu(X @ W1 + b1) @ W2) * combine_scale
    All GEMMs run in bf16 on the PE with fp32 PSUM accumulation; gelu (exact,
    erf-based) is fused into the PSUM eviction on the scalar engine; the
    combine-weight scaling is fused into the second GEMM's PSUM eviction on
    the vector engine.
  - Host scatters per-expert outputs back (indices are unique per expert) and
    adds the (gate-weighted) b2 term.

Layouts (per core):
  xt  [H, C]  bf16   gathered tokens, transposed (contraction dim on partitions)
  w1  [H, FF] bf16   natural layout == lhsT for GEMM1
  w2  [FF, H] bf16   natural layout == rhs for GEMM2
  b1p [128, FF/128]  f32, column ff = b1[ff*128:(ff+1)*128]
  scp [128, C/128]   f32, column j = combine scale of tokens j*128..j*128+127
  y   [C, H]  f32    per-slot FFN output, already scaled by combine weight

GEMM1 produces Hmid^T (FF on partitions) so GEMM2 can consume it as the
stationary operand with zero transposes anywhere.
"""

import sys

if "/opt/trn_rl_repo" not in sys.path:
    sys.path.insert(0, "/opt/trn_rl_repo")

import numpy as np
import ml_dtypes

H = 1024          # hidden size
E = 8             # experts == cores
TOPK = 2
FF = 4 * H        # expert hidden dim
P = 128           # SBUF partitions
NB = 384          # token block (matmul free dim, <= 512 PSUM bank)
NH = 512          # GEMM2 output free-dim tile

_prog_cache: dict[int, object] = {}
LAST_RESULTS = None  # BassKernelResults of the most recent run (for test harness)
TRACE = False        # test harness can set kernel.TRACE = True for profiling
ACT_OVERRIDE = None  # sim-only: CoreSim lacks Gelu; tests may set e.g. "Relu"
LAST_CALL = None     # (nc, in_maps) of the most recent run, for re-runs


def _build_program(C: int, A: int | None = None):
    """Build + compile the per-core SPMD Bass program.

    C: padded capacity (multiple of 128) — the DRAM/SBUF array width.
    A: active slot count (<= C) — compute covers only slots [0, A);
       y rows [A, C) stay at the runtime's zero-fill.
    """
    from contextlib import ExitStack

    from concourse import bacc
    import concourse.mybir as mybir
    import concourse.tile as tile

    if A is None:
        A = C
    dt = mybir.dt
    KH = H // P            # 8  contraction chunks for GEMM1
    KF = FF // P           # 32 contraction chunks for GEMM2
    assert C % P == 0 and 0 < A <= C
    # token blocks over the active range: full NB blocks plus a remainder
    blocks = []
    t = 0
    while A - t >= NB:
        blocks.append((t, NB))
        t += NB
    if t < A:
        blocks.append((t, A - t))

    nc = bacc.Bacc(None, target_bir_lowering=False, debug=False)

    xt = nc.dram_tensor("xt", [H, C], dt.bfloat16, kind="ExternalInput")
    w1 = nc.dram_tensor("w1", [H, FF], dt.bfloat16, kind="ExternalInput")
    w2 = nc.dram_tensor("w2", [FF, H], dt.bfloat16, kind="ExternalInput")
    b1p = nc.dram_tensor("b1p", [P, KF], dt.float32, kind="ExternalInput")
    scp = nc.dram_tensor("scp", [P, C // P], dt.float32, kind="ExternalInput")
    y = nc.dram_tensor("y", [C, H], dt.float32, kind="ExternalOutput")

    with ExitStack() as ctx:
        tc = ctx.enter_context(tile.TileContext(nc))
        wpool = ctx.enter_context(tc.tile_pool(name="wpool", bufs=1))
        hpool = ctx.enter_context(tc.tile_pool(name="hpool", bufs=1))
        psA = ctx.enter_context(tc.tile_pool(name="psA", bufs=3, space="PSUM"))
        psB = ctx.enter_context(tc.tile_pool(name="psB", bufs=3, space="PSUM"))
        opool = ctx.enter_context(tc.tile_pool(name="opool", bufs=4))

        # --- resident inputs ------------------------------------------------
        # DMA pieces match compute-consumption granularity so the first
        # matmul group's dependencies are a ~2MB critical prefix, not the
        # whole 19MB: xt per (k, block), W1 per (k, ff-block). Emission
        # order = priority: block-0 xt + ff-block-0 W1 first, then the
        # rest, then W2 (not needed until GEMM2, ~halfway in).
        xtile = wpool.tile([P, KH, C], dt.bfloat16, tag="xtile", name="xtile")
        w1t = wpool.tile([P, KH, FF], dt.bfloat16, tag="w1t", name="w1t")
        FB = 4
        FBW = FF // FB
        for k in range(KH):
            t0, nb = blocks[0]
            nc.sync.dma_start(
                out=xtile[:, k, t0:t0 + nb], in_=xt[k * P:(k + 1) * P, t0:t0 + nb]
            )
        for k in range(KH):
            nc.sync.dma_start(
                out=w1t[:, k, 0:FBW], in_=w1[k * P:(k + 1) * P, 0:FBW]
            )
        for t0, nb in blocks[1:]:
            for k in range(KH):
                nc.sync.dma_start(
                    out=xtile[:, k, t0:t0 + nb],
                    in_=xt[k * P:(k + 1) * P, t0:t0 + nb],
                )
        b1t = wpool.tile([P, KF], dt.float32, tag="b1t", name="b1t")
        nc.sync.dma_start(out=b1t[:], in_=b1p[:])
        sct = wpool.tile([P, C // P], dt.float32, tag="sct", name="sct")
        nc.sync.dma_start(out=sct[:], in_=scp[:])
        for fb in range(1, FB):
            for k in range(KH):
                nc.sync.dma_start(
                    out=w1t[:, k, fb * FBW:(fb + 1) * FBW],
                    in_=w1[k * P:(k + 1) * P, fb * FBW:(fb + 1) * FBW],
                )
        w2t = wpool.tile([P, KF, H], dt.bfloat16, tag="w2t", name="w2t")
        for k in range(KF):
            nc.sync.dma_start(out=w2t[:, k, :], in_=w2[k * P:(k + 1) * P, :])

        # --- main loop over token blocks ------------------------------------
        act = getattr(mybir.ActivationFunctionType, ACT_OVERRIDE or "Gelu")
        for t0, nb in blocks:
            # GEMM1: HmidT[ff, t] = gelu(sum_h W1[h, ff] * xt[h, t] + b1[ff])
            hblk = hpool.tile([P, KF, NB], dt.bfloat16, tag="hblk", name="hblk")
            for ff in range(KF):
                pa = psA.tile([P, NB], dt.float32, tag="pa", name="pa")
                for k in range(KH):
                    nc.tensor.matmul(
                        pa[:, :nb],
                        lhsT=w1t[:, k, ff * P:(ff + 1) * P],
                        rhs=xtile[:, k, t0:t0 + nb],
                        start=(k == 0),
                        stop=(k == KH - 1),
                    )
                nc.scalar.activation(
                    hblk[:, ff, :nb],
                    pa[:, :nb],
                    act,
                    bias=b1t[:, ff:ff + 1],
                )
            # GEMM2: Y[t, h] = (sum_f HmidT[f, t] * W2[f, h]) * scale[t]
            for tt0 in range(0, nb, P):
                m = min(P, nb - tt0)
                gt = (t0 + tt0) // P
                for nh in range(H // NH):
                    pb = psB.tile([P, NH], dt.float32, tag="pb", name="pb")
                    for k in range(KF):
                        nc.tensor.matmul(
                            pb[:m],
                            lhsT=hblk[:, k, tt0:tt0 + m],
                            rhs=w2t[:, k, nh * NH:(nh + 1) * NH],
                            start=(k == 0),
                            stop=(k == KF - 1),
                        )
                    ot = opool.tile([P, NH], dt.float32, tag="ot", name="ot")
                    nc.vector.tensor_scalar_mul(ot[:m], pb[:m], sct[:m, gt:gt + 1])
                    nc.sync.dma_start(
                        out=y[gt * P:gt * P + m, nh * NH:(nh + 1) * NH],
                        in_=ot[:m],
                    )

    nc.compile()
    return nc


def _get_program(C: int, A: int | None = None):
    key = (C, A)
    if key not in _prog_cache:
        _prog_cache[key] = _build_program(C, A)
    return _prog_cache[key]


def _route(xf: np.ndarray, Wg: np.ndarray, bg: np.ndarray):
    """fp32 gate: softmax -> top-2 (stable order, matches jax top_k) -> renorm."""
    logits = xf @ np.asarray(Wg, np.float32) + np.asarray(bg, np.float32)
    m = logits.max(axis=1, keepdims=True)
    p = np.exp(logits - m, dtype=np.float32)
    p /= p.sum(axis=1, keepdims=True)
    order = np.argsort(-p, axis=1, kind="stable")
    idx = order[:, :TOPK]
    pv = np.take_along_axis(p, idx, axis=1)
    vals = (pv / pv.sum(axis=1, keepdims=True)).astype(np.float32)
    return idx, vals


def kernel(x, Wg, bg, W1, b1, W2, b2):
    global LAST_RESULTS
    from concourse.bass_utils import run_bass_kernel_spmd

    x = np.asarray(x, np.float32)
    xf = x.reshape(-1, H)
    T = xf.shape[0]

    idx, vals = _route(xf, Wg, bg)

    counts = np.bincount(idx.ravel(), minlength=E)
    A = int(counts.max())
    C = max(P, -(-A // P) * P)

    nc = _get_program(C, A)

    bf16 = ml_dtypes.bfloat16
    W1 = np.asarray(W1, np.float32)
    W2 = np.asarray(W2, np.float32)
    b1 = np.asarray(b1, np.float32)
    KF = FF // P

    in_maps = []
    ids_list = []
    for e in range(E):
        sel = idx == e                      # [T, 2]; at most one True per row
        ids = np.nonzero(sel.any(axis=1))[0]
        sc = vals[sel]                      # row-major => aligned with ids
        cnt = ids.shape[0]

        xe = np.zeros((C, H), np.float32)
        xe[:cnt] = xf[ids]
        scpad = np.zeros((C,), np.float32)
        scpad[:cnt] = sc

        in_maps.append({
            "xt": np.ascontiguousarray(xe.T).astype(bf16),
            "w1": W1[e].astype(bf16),
            "w2": W2[e].astype(bf16),
            "b1p": np.ascontiguousarray(b1[e].reshape(KF, P).T),
            "scp": np.ascontiguousarray(scpad.reshape(C // P, P).T),
        })
        ids_list.append(ids)

    global LAST_CALL
    LAST_CALL = (nc, in_maps)
    LAST_RESULTS = run_bass_kernel_spmd(nc, in_maps, list(range(E)), trace=TRACE)

    out = np.zeros((T, H), np.float32)
    for e in range(E):
        ids = ids_list[e]
        out[ids] += LAST_RESULTS.results[e]["y"][: ids.shape[0]]

    b2 = np.asarray(b2, np.float32)
    out += vals[:, 0:1] * b2[idx[:, 0]] + vals[:, 1:2] * b2[idx[:, 1]]
    return out.reshape(x.shape)


# revision 16
# speedup vs baseline: 1.0552x; 1.0552x over previous
"""Expert-parallel MoE FFN kernel for Trainium2 (8 NeuronCores).

Strategy (sharding_hint: expert-parallel):
  - Host computes the gate in fp32 (softmax -> top-2 -> renormalize) and
    dispatches tokens to experts (the "all-to-all" happens in host staging,
    which is legal because kernel() receives FULL inputs).
  - Core e holds expert e's weights (bf16) and processes its gathered tokens
    (padded to a static capacity C) through the FFN:
        Y = (gelu(X @ W1 + b1) @ W2) * combine_scale
    All GEMMs run in bf16 on the PE with fp32 PSUM accumulation; gelu (exact,
    erf-based) is fused into the PSUM eviction on the scalar engine; the
    combine-weight scaling is fused into the second GEMM's PSUM eviction on
    the vector engine.
  - Host scatters per-expert outputs back (indices are unique per expert) and
    adds the (gate-weighted) b2 term.

Layouts (per core):
  xt  [H, C]  bf16   gathered tokens, transposed (contraction dim on partitions)
  w1  [H, FF] bf16   natural layout == lhsT for GEMM1
  w2  [FF, H] bf16   natural layout == rhs for GEMM2
  b1p [128, FF/128]  f32, column ff = b1[ff*128:(ff+1)*128]
  scp [128, C/128]   f32, column j = combine scale of tokens j*128..j*128+127
  y   [C, H]  f32    per-slot FFN output, already scaled by combine weight

GEMM1 produces Hmid^T (FF on partitions) so GEMM2 can consume it as the
stationary operand with zero transposes anywhere.
"""

import sys

if "/opt/trn_rl_repo" not in sys.path:
    sys.path.insert(0, "/opt/trn_rl_repo")

import numpy as np
import ml_dtypes

H = 1024          # hidden size
E = 8             # experts == cores
TOPK = 2
FF = 4 * H        # expert hidden dim
P = 128           # SBUF partitions
NB = 384          # token block (matmul free dim, <= 512 PSUM bank)
NH = 512          # GEMM2 output free-dim tile

_prog_cache: dict[int, object] = {}
LAST_RESULTS = None  # BassKernelResults of the most recent run (for test harness)
TRACE = False        # test harness can set kernel.TRACE = True for profiling
ACT_OVERRIDE = None  # sim-only: CoreSim lacks Gelu; tests may set e.g. "Relu"
LAST_CALL = None     # (nc, in_maps) of the most recent run, for re-runs


def _build_program(C: int, A: int | None = None):
    """Build + compile the per-core SPMD Bass program.

    C: padded capacity (multiple of 128) — the DRAM/SBUF array width.
    A: active slot count (<= C) — compute covers only slots [0, A);
       y rows [A, C) stay at the runtime's zero-fill.
    """
    from contextlib import ExitStack

    from concourse import bacc
    import concourse.mybir as mybir
    import concourse.tile as tile

    if A is None:
        A = C
    dt = mybir.dt
    KH = H // P            # 8  contraction chunks for GEMM1
    KF = FF // P           # 32 contraction chunks for GEMM2
    assert C % P == 0 and 0 < A <= C
    # token blocks over the active range: full NB blocks plus a remainder
    blocks = []
    t = 0
    while A - t >= NB:
        blocks.append((t, NB))
        t += NB
    if t < A:
        blocks.append((t, A - t))

    nc = bacc.Bacc(None, target_bir_lowering=False, debug=False)

    xt = nc.dram_tensor("xt", [H, C], dt.bfloat16, kind="ExternalInput")
    w1 = nc.dram_tensor("w1", [H, FF], dt.bfloat16, kind="ExternalInput")
    w2 = nc.dram_tensor("w2", [FF, H], dt.bfloat16, kind="ExternalInput")
    b1p = nc.dram_tensor("b1p", [P, KF], dt.float32, kind="ExternalInput")
    scp = nc.dram_tensor("scp", [P, C // P], dt.float32, kind="ExternalInput")
    y = nc.dram_tensor("y", [C, H], dt.float32, kind="ExternalOutput")

    with ExitStack() as ctx:
        tc = ctx.enter_context(tile.TileContext(nc))
        wpool = ctx.enter_context(tc.tile_pool(name="wpool", bufs=1))
        hpool = ctx.enter_context(tc.tile_pool(name="hpool", bufs=1))
        psA = ctx.enter_context(tc.tile_pool(name="psA", bufs=3, space="PSUM"))
        psB = ctx.enter_context(tc.tile_pool(name="psB", bufs=3, space="PSUM"))
        opool = ctx.enter_context(tc.tile_pool(name="opool", bufs=4))

        # --- resident inputs ------------------------------------------------
        # Few large multi-chunk DMAs: descriptor ISSUE on the sync engine
        # (~0.7us per dma_start) is the startup bottleneck, while one big
        # DMA fans out across all 16 SDMA engines at full fabric BW.
        # Emission order = consumption order: biases, block-0 tokens,
        # W1 ff-blocks (GEMM1 eats one ff-block per ~10us), W2 (needed at
        # ~50% mark), remaining token blocks (needed at ~35%... later).
        xtile = wpool.tile([P, KH, C], dt.bfloat16, tag="xtile", name="xtile")
        w1t = wpool.tile([P, KH, FF], dt.bfloat16, tag="w1t", name="w1t")
        w2t = wpool.tile([P, KF, H], dt.bfloat16, tag="w2t", name="w2t")
        xt_r = xt[:, :].rearrange("(k p) t -> p k t", p=P)
        w1_r = w1[:, :].rearrange("(k p) f -> p k f", p=P)
        w2_r = w2[:, :].rearrange("(k p) h -> p k h", p=P)

        b1t = wpool.tile([P, KF], dt.float32, tag="b1t", name="b1t")
        nc.sync.dma_start(out=b1t[:], in_=b1p[:])
        sct = wpool.tile([P, C // P], dt.float32, tag="sct", name="sct")
        nc.sync.dma_start(out=sct[:], in_=scp[:])

        nb0 = blocks[0][1]
        nc.sync.dma_start(out=xtile[:, :, 0:nb0], in_=xt_r[:, :, 0:nb0])
        FB = 4
        FBW = FF // FB
        for fb in range(FB):
            nc.sync.dma_start(
                out=w1t[:, :, fb * FBW:(fb + 1) * FBW],
                in_=w1_r[:, :, fb * FBW:(fb + 1) * FBW],
            )
        nc.sync.dma_start(out=w2t[:, :, :], in_=w2_r[:, :, :])
        if A > nb0:
            nc.sync.dma_start(out=xtile[:, :, nb0:A], in_=xt_r[:, :, nb0:A])

        # --- main loop over token blocks ------------------------------------
        act = getattr(mybir.ActivationFunctionType, ACT_OVERRIDE or "Gelu")
        for t0, nb in blocks:
            # GEMM1: HmidT[ff, t] = gelu(sum_h W1[h, ff] * xt[h, t] + b1[ff])
            hblk = hpool.tile([P, KF, NB], dt.bfloat16, tag="hblk", name="hblk")
            nbp = -(-nb // P) * P  # phase-B token-tile grid (full 128s)
            if nbp > nb:
                # pad columns feed the (ignored, scale-0) tail rows of the
                # last GEMM2 tile; zero them so nothing reads uninit SBUF
                nc.gpsimd.memset(hblk[:, :, nb:nbp], 0.0)
            for ff in range(KF):
                pa = psA.tile([P, NB], dt.float32, tag="pa", name="pa")
                for k in range(KH):
                    nc.tensor.matmul(
                        pa[:, :nb],
                        lhsT=w1t[:, k, ff * P:(ff + 1) * P],
                        rhs=xtile[:, k, t0:t0 + nb],
                        start=(k == 0),
                        stop=(k == KH - 1),
                    )
                nc.scalar.activation(
                    hblk[:, ff, :nb],
                    pa[:, :nb],
                    act,
                    bias=b1t[:, ff:ff + 1],
                )
            # GEMM2: Y[t, h] = (sum_f HmidT[f, t] * W2[f, h]) * scale[t]
            # Token tiles are always full 128 rows: rows past A are pad
            # slots (combine scale 0, ignored by the host), which keeps
            # the output DMA a single contiguous-full-row transfer.
            for tt0 in range(0, nbp, P):
                gt = (t0 + tt0) // P
                ot = opool.tile([P, H], dt.float32, tag="ot", name="ot")
                for nh in range(H // NH):
                    pb = psB.tile([P, NH], dt.float32, tag="pb", name="pb")
                    for k in range(KF):
                        nc.tensor.matmul(
                            pb[:],
                            lhsT=hblk[:, k, tt0:tt0 + P],
                            rhs=w2t[:, k, nh * NH:(nh + 1) * NH],
                            start=(k == 0),
                            stop=(k == KF - 1),
                        )
                    nc.vector.tensor_scalar_mul(
                        ot[:, nh * NH:(nh + 1) * NH], pb[:], sct[:, gt:gt + 1]
                    )
                nc.sync.dma_start(out=y[gt * P:(gt + 1) * P, :], in_=ot[:])

    nc.compile()
    return nc


def _get_program(C: int, A: int | None = None):
    key = (C, A)
    if key not in _prog_cache:
        _prog_cache[key] = _build_program(C, A)
    return _prog_cache[key]


def _route(xf: np.ndarray, Wg: np.ndarray, bg: np.ndarray):
    """fp32 gate: softmax -> top-2 (stable order, matches jax top_k) -> renorm."""
    logits = xf @ np.asarray(Wg, np.float32) + np.asarray(bg, np.float32)
    m = logits.max(axis=1, keepdims=True)
    p = np.exp(logits - m, dtype=np.float32)
    p /= p.sum(axis=1, keepdims=True)
    order = np.argsort(-p, axis=1, kind="stable")
    idx = order[:, :TOPK]
    pv = np.take_along_axis(p, idx, axis=1)
    vals = (pv / pv.sum(axis=1, keepdims=True)).astype(np.float32)
    return idx, vals


def kernel(x, Wg, bg, W1, b1, W2, b2):
    global LAST_RESULTS
    from concourse.bass_utils import run_bass_kernel_spmd

    x = np.asarray(x, np.float32)
    xf = x.reshape(-1, H)
    T = xf.shape[0]

    idx, vals = _route(xf, Wg, bg)

    counts = np.bincount(idx.ravel(), minlength=E)
    A = int(counts.max())
    C = max(P, -(-A // P) * P)

    nc = _get_program(C, A)

    bf16 = ml_dtypes.bfloat16
    W1 = np.asarray(W1, np.float32)
    W2 = np.asarray(W2, np.float32)
    b1 = np.asarray(b1, np.float32)
    KF = FF // P

    in_maps = []
    ids_list = []
    for e in range(E):
        sel = idx == e                      # [T, 2]; at most one True per row
        ids = np.nonzero(sel.any(axis=1))[0]
        sc = vals[sel]                      # row-major => aligned with ids
        cnt = ids.shape[0]

        xe = np.zeros((C, H), np.float32)
        xe[:cnt] = xf[ids]
        scpad = np.zeros((C,), np.float32)
        scpad[:cnt] = sc

        in_maps.append({
            "xt": np.ascontiguousarray(xe.T).astype(bf16),
            "w1": W1[e].astype(bf16),
            "w2": W2[e].astype(bf16),
            "b1p": np.ascontiguousarray(b1[e].reshape(KF, P).T),
            "scp": np.ascontiguousarray(scpad.reshape(C // P, P).T),
        })
        ids_list.append(ids)

    global LAST_CALL
    LAST_CALL = (nc, in_maps)
    LAST_RESULTS = run_bass_kernel_spmd(nc, in_maps, list(range(E)), trace=TRACE)

    out = np.zeros((T, H), np.float32)
    for e in range(E):
        ids = ids_list[e]
        out[ids] += LAST_RESULTS.results[e]["y"][: ids.shape[0]]

    b2 = np.asarray(b2, np.float32)
    out += vals[:, 0:1] * b2[idx[:, 0]] + vals[:, 1:2] * b2[idx[:, 1]]
    return out.reshape(x.shape)


# revision 23
# speedup vs baseline: 1.0943x; 1.0370x over previous
"""Expert-parallel MoE FFN kernel for Trainium2 (8 NeuronCores).

Strategy (sharding_hint: expert-parallel):
  - Host computes the gate in fp32 (softmax -> top-2 -> renormalize) and
    dispatches tokens to experts (the "all-to-all" happens in host staging,
    which is legal because kernel() receives FULL inputs).
  - Core e holds expert e's weights (bf16) and processes its gathered tokens
    (padded to a static capacity C) through the FFN:
        Y = (gelu(X @ W1 + b1) @ W2) * combine_scale
    All GEMMs run in bf16 on the PE with fp32 PSUM accumulation; gelu (exact,
    erf-based) is fused into the PSUM eviction on the scalar engine; the
    combine-weight scaling is fused into the second GEMM's PSUM eviction on
    the vector engine.
  - Host scatters per-expert outputs back (indices are unique per expert) and
    adds the (gate-weighted) b2 term.

Layouts (per core):
  xt  [H, C]  bf16   gathered tokens, transposed (contraction dim on partitions)
  w1  [H, FF] bf16   natural layout == lhsT for GEMM1
  w2  [FF, H] bf16   natural layout == lhsT for GEMM2 (stationary)
  b1p [128, FF/128]  f32, column ff = b1[ff*128:(ff+1)*128]
  y   [H, C]  f32    transposed per-slot FFN output (unscaled)

GEMM1 produces Hmid^T (FF on partitions); GEMM2 keeps tokens on the moving
operand (cycles scale with the exact token count, not 128-padded tiles) and
produces Y^T. The combine-weight scale and the final transpose happen on the
host during the scatter — zero transposes or gather/scatter on device.
"""

import sys

if "/opt/trn_rl_repo" not in sys.path:
    sys.path.insert(0, "/opt/trn_rl_repo")

import numpy as np
import ml_dtypes

H = 1024          # hidden size
E = 8             # experts == cores
TOPK = 2
FF = 4 * H        # expert hidden dim
P = 128           # SBUF partitions
NB = 384          # token block (matmul free dim, <= 512 PSUM bank)
NH = 512          # GEMM2 output free-dim tile

_prog_cache: dict[int, object] = {}
LAST_RESULTS = None  # BassKernelResults of the most recent run (for test harness)
TRACE = False        # test harness can set kernel.TRACE = True for profiling
ACT_OVERRIDE = None  # sim-only: CoreSim lacks Gelu; tests may set e.g. "Relu"
LAST_CALL = None     # (nc, in_maps) of the most recent run, for re-runs


def _build_program(C: int, A: int | None = None):
    """Build + compile the per-core SPMD Bass program.

    C: padded capacity (multiple of 128) — the DRAM/SBUF array width.
    A: active slot count (<= C) — compute covers only slots [0, A);
       y rows [A, C) stay at the runtime's zero-fill.
    """
    from contextlib import ExitStack

    from concourse import bacc
    import concourse.mybir as mybir
    import concourse.tile as tile

    if A is None:
        A = C
    dt = mybir.dt
    KH = H // P            # 8  contraction chunks for GEMM1
    KF = FF // P           # 32 contraction chunks for GEMM2
    assert C % P == 0 and 0 < A <= C
    # token blocks over the active range: full NB blocks plus a remainder
    blocks = []
    t = 0
    while A - t >= NB:
        blocks.append((t, NB))
        t += NB
    if t < A:
        blocks.append((t, A - t))

    nc = bacc.Bacc(None, target_bir_lowering=False, debug=False)

    xt = nc.dram_tensor("xt", [H, C], dt.bfloat16, kind="ExternalInput")
    w1 = nc.dram_tensor("w1", [H, FF], dt.bfloat16, kind="ExternalInput")
    w2 = nc.dram_tensor("w2", [FF, H], dt.bfloat16, kind="ExternalInput")
    b1p = nc.dram_tensor("b1p", [P, KF], dt.float32, kind="ExternalInput")
    y = nc.dram_tensor("y", [H, C], dt.float32, kind="ExternalOutput")

    with ExitStack() as ctx:
        tc = ctx.enter_context(tile.TileContext(nc))
        wpool = ctx.enter_context(tc.tile_pool(name="wpool", bufs=1))
        hpool = ctx.enter_context(tc.tile_pool(name="hpool", bufs=1))
        psA = ctx.enter_context(tc.tile_pool(name="psA", bufs=3, space="PSUM"))
        psB = ctx.enter_context(tc.tile_pool(name="psB", bufs=3, space="PSUM"))
        opool = ctx.enter_context(tc.tile_pool(name="opool", bufs=4))

        # --- resident inputs ------------------------------------------------
        # Few large multi-chunk DMAs: descriptor ISSUE on the sync engine
        # (~0.7us per dma_start) is the startup bottleneck, while one big
        # DMA fans out across all 16 SDMA engines at full fabric BW.
        # Emission order = consumption order: biases, block-0 tokens,
        # W1 ff-blocks (GEMM1 eats one ff-block per ~10us), W2 (needed at
        # ~50% mark), remaining token blocks (needed at ~35%... later).
        xtile = wpool.tile([P, KH, C], dt.bfloat16, tag="xtile", name="xtile")
        w1t = wpool.tile([P, KH, FF], dt.bfloat16, tag="w1t", name="w1t")
        w2t = wpool.tile([P, KF, H], dt.bfloat16, tag="w2t", name="w2t")
        xt_r = xt[:, :].rearrange("(k p) t -> p k t", p=P)
        w1_r = w1[:, :].rearrange("(k p) f -> p k f", p=P)
        w2_r = w2[:, :].rearrange("(k p) h -> p k h", p=P)

        b1t = wpool.tile([P, KF], dt.float32, tag="b1t", name="b1t")
        nc.sync.dma_start(out=b1t[:], in_=b1p[:])

        nb0 = blocks[0][1]
        nc.sync.dma_start(out=xtile[:, :, 0:nb0], in_=xt_r[:, :, 0:nb0])
        # first W1 piece halved so the first matmul group's critical DMA
        # prefix (xt block 0 + W1 for ff-tiles 0-3) is ~1.8MB
        w1_edges = [0, FF // 8, FF // 4, FF // 2, 3 * FF // 4, FF]
        for fb in range(len(w1_edges) - 1):
            nc.sync.dma_start(
                out=w1t[:, :, w1_edges[fb]:w1_edges[fb + 1]],
                in_=w1_r[:, :, w1_edges[fb]:w1_edges[fb + 1]],
            )
        nc.sync.dma_start(out=w2t[:, :, :], in_=w2_r[:, :, :])
        if A > nb0:
            nc.sync.dma_start(out=xtile[:, :, nb0:A], in_=xt_r[:, :, nb0:A])

        # --- main loop over token blocks ------------------------------------
        act = getattr(mybir.ActivationFunctionType, ACT_OVERRIDE or "Gelu")
        for t0, nb in blocks:
            # GEMM1: HmidT[ff, t] = gelu(sum_h W1[h, ff] * xt[h, t] + b1[ff])
            hblk = hpool.tile([P, KF, NB], dt.bfloat16, tag="hblk", name="hblk")
            for ff in range(KF):
                pa = psA.tile([P, NB], dt.float32, tag="pa", name="pa")
                for k in range(KH):
                    nc.tensor.matmul(
                        pa[:, :nb],
                        lhsT=w1t[:, k, ff * P:(ff + 1) * P],
                        rhs=xtile[:, k, t0:t0 + nb],
                        start=(k == 0),
                        stop=(k == KH - 1),
                    )
                nc.scalar.activation(
                    hblk[:, ff, :nb],
                    pa[:, :nb],
                    act,
                    bias=b1t[:, ff:ff + 1],
                )
            # GEMM2: YT[h, t] = sum_f W2[f, h] * HmidT[f, t]
            # W2 chunks are the stationary operand; tokens stay on the
            # moving side so cycles scale with the exact token count.
            for ht in range(H // P):
                pb = psB.tile([P, NB], dt.float32, tag="pb", name="pb")
                for k in range(KF):
                    nc.tensor.matmul(
                        pb[:, :nb],
                        lhsT=w2t[:, k, ht * P:(ht + 1) * P],
                        rhs=hblk[:, k, :nb],
                        start=(k == 0),
                        stop=(k == KF - 1),
                    )
                ot = opool.tile([P, NB], dt.float32, tag="ot", name="ot")
                nc.vector.tensor_copy(ot[:, :nb], pb[:, :nb])
                nc.sync.dma_start(
                    out=y[ht * P:(ht + 1) * P, t0:t0 + nb], in_=ot[:, :nb]
                )

    nc.compile()
    return nc


def _get_program(C: int, A: int | None = None):
    key = (C, A)
    if key not in _prog_cache:
        _prog_cache[key] = _build_program(C, A)
    return _prog_cache[key]


def _route(xf: np.ndarray, Wg: np.ndarray, bg: np.ndarray):
    """fp32 gate: softmax -> top-2 (stable order, matches jax top_k) -> renorm."""
    logits = xf @ np.asarray(Wg, np.float32) + np.asarray(bg, np.float32)
    m = logits.max(axis=1, keepdims=True)
    p = np.exp(logits - m, dtype=np.float32)
    p /= p.sum(axis=1, keepdims=True)
    order = np.argsort(-p, axis=1, kind="stable")
    idx = order[:, :TOPK]
    pv = np.take_along_axis(p, idx, axis=1)
    vals = (pv / pv.sum(axis=1, keepdims=True)).astype(np.float32)
    return idx, vals


def kernel(x, Wg, bg, W1, b1, W2, b2):
    global LAST_RESULTS
    from concourse.bass_utils import run_bass_kernel_spmd

    x = np.asarray(x, np.float32)
    xf = x.reshape(-1, H)
    T = xf.shape[0]

    idx, vals = _route(xf, Wg, bg)

    counts = np.bincount(idx.ravel(), minlength=E)
    A = int(counts.max())
    C = max(P, -(-A // P) * P)

    nc = _get_program(C, A)

    bf16 = ml_dtypes.bfloat16
    W1 = np.asarray(W1, np.float32)
    W2 = np.asarray(W2, np.float32)
    b1 = np.asarray(b1, np.float32)
    KF = FF // P

    in_maps = []
    ids_list = []
    for e in range(E):
        sel = idx == e                      # [T, 2]; at most one True per row
        ids = np.nonzero(sel.any(axis=1))[0]
        sc = vals[sel]                      # row-major => aligned with ids
        cnt = ids.shape[0]

        xe = np.zeros((C, H), np.float32)
        xe[:cnt] = xf[ids]

        in_maps.append({
            "xt": np.ascontiguousarray(xe.T).astype(bf16),
            "w1": W1[e].astype(bf16),
            "w2": W2[e].astype(bf16),
            "b1p": np.ascontiguousarray(b1[e].reshape(KF, P).T),
        })
        ids_list.append((ids, sc))

    global LAST_CALL
    LAST_CALL = (nc, in_maps)
    LAST_RESULTS = run_bass_kernel_spmd(nc, in_maps, list(range(E)), trace=TRACE)

    out = np.zeros((T, H), np.float32)
    for e in range(E):
        ids, sc = ids_list[e]
        yt = LAST_RESULTS.results[e]["y"]          # [H, C], unscaled
        out[ids] += yt[:, : ids.shape[0]].T * sc[:, None]

    b2 = np.asarray(b2, np.float32)
    out += vals[:, 0:1] * b2[idx[:, 0]] + vals[:, 1:2] * b2[idx[:, 1]]
    return out.reshape(x.shape)
